# revision 4
# baseline (speedup 1.0000x reference)
"""CirLinear Trainium2 kernel: y = x @ build_weight(W, alphas, gumbels)^T + bias.

Strategy (8 NeuronCores):
 - data-parallel shard of x over tokens (16384 -> 2048/core)
 - circulant weight build sharded over out_features rows (256 rows/core),
   AllGather of the combined weight (bf16), per-core transposed reload
 - bf16 matmul with fp32 PSUM accumulation, fp32 bias add, fp32 output
"""
import sys

sys.path.insert(0, '/opt/trn_rl_repo')

import numpy as np

import concourse.bass as bass
from concourse import bacc
import concourse.mybir as mybir
from concourse.tile import TileContext
from concourse.bass_utils import run_bass_kernel_spmd

N_CORES = 8
BATCH, TOKENS, IN_F, OUT_F = 16, 1024, 2048, 2048
TOK_TOTAL = BATCH * TOKENS            # 16384
TOK = TOK_TOTAL // N_CORES            # 2048 tokens per core
ROWS = OUT_F // N_CORES               # 256 weight rows built per core
NQ64 = ROWS // 64                     # 4 q64 blocks per core
SCALES = [2, 4, 8, 16, 32, 64]
N_IC = IN_F // 128                    # 16 contraction chunks
N_TG = TOK // 512                     # 4 token groups of 512
N_O = OUT_F // 512                    # 4 output-column tiles of 512

bf16 = mybir.dt.bfloat16
f32 = mybir.dt.float32

_CACHE = {}


def _build_nc():
    nc = bacc.Bacc("TRN2", target_bir_lowering=False, debug=False, num_devices=N_CORES)
    xs = nc.dram_tensor("xs", [TOK, IN_F], f32, kind="ExternalInput")
    ws = nc.dram_tensor("ws", [ROWS, IN_F], f32, kind="ExternalInput")
    bias = nc.dram_tensor("bias", [1, OUT_F], f32, kind="ExternalInput")
    alphas = nc.dram_tensor("alphas", [1, 7], f32, kind="ExternalInput")
    gumbels = nc.dram_tensor("gumbels", [1, 7], f32, kind="ExternalInput")
    out = nc.dram_tensor("out", [TOK, OUT_F], f32, kind="ExternalOutput")

    w_loc = nc.dram_tensor("w_loc", [ROWS, IN_F], bf16)
    w_full = nc.dram_tensor("w_full", [OUT_F, IN_F], bf16, addr_space="Shared")
    xbf = nc.dram_tensor("xbf", [TOK, IN_F], bf16)

    with TileContext(nc) as tc:
        # ---------- softmax(alphas + gumbels) broadcast to 128 partitions ----------
        asb = nc.alloc_sbuf_tensor("asb", [128, 7], f32).ap()
        gsb = nc.alloc_sbuf_tensor("gsb", [128, 7], f32).ap()
        a_bc = nc.alloc_sbuf_tensor("a_bc", [128, 7], f32).ap()
        ssum = nc.alloc_sbuf_tensor("ssum", [128, 1], f32).ap()
        nc.gpsimd.dma_start(out=asb, in_=bass.AP(tensor=alphas, offset=0, ap=[[0, 128], [1, 7]]))
        nc.gpsimd.dma_start(out=gsb, in_=bass.AP(tensor=gumbels, offset=0, ap=[[0, 128], [1, 7]]))
        nc.vector.tensor_tensor(out=asb, in0=asb, in1=gsb, op=mybir.AluOpType.add)
        nc.scalar.activation(out=asb, in_=asb, func=mybir.ActivationFunctionType.Exp)
        nc.vector.tensor_reduce(out=ssum, in_=asb, axis=mybir.AxisListType.X, op=mybir.AluOpType.add)
        nc.vector.reciprocal(out=ssum, in_=ssum)
        nc.vector.tensor_scalar_mul(a_bc, asb, ssum)

        # ---------- bias broadcast ----------
        bias_bc = nc.alloc_sbuf_tensor("bias_bc", [128, OUT_F], f32).ap()
        nc.gpsimd.dma_start(out=bias_bc, in_=bass.AP(tensor=bias, offset=0, ap=[[0, 128], [1, OUT_F]]))

        # ---------- circulant weight build: local 256 rows in 64-block layout ----------
        # partition = (q64, p64) : 4*32 = 128 ; free = (r64, s64) : 64*64 = 4096
        wb = nc.alloc_sbuf_tensor("wb", [128, 4096], bf16).ap()
        ws_4d = ws.ap().rearrange("(q r) (p s) -> q p r s", r=64, s=64)
        for q in range(NQ64):
            nc.gpsimd.dma_start(out=wb[q * 32:(q + 1) * 32, :], in_=ws_4d[q])

        acc = nc.alloc_sbuf_tensor("acc", [128, 4096], f32).ap()
        wbpad = nc.alloc_sbuf_tensor("wbpad", [128, 8192], bf16).ap()
        d_raw = nc.alloc_sbuf_tensor("d_raw", [128, 2048], f32).ap()
        dpad = nc.alloc_sbuf_tensor("dpad", [128, 4096], f32).ap()

        nc.vector.tensor_scalar_mul(acc, wb, a_bc[:, 0:1])

        def sb(t, off, dims):
            return bass.AP(tensor=t.tensor, offset=off, ap=[list(t.ap[0])] + dims)

        for idx, b in enumerate(SCALES, start=1):
            nv = 64 // b
            src = sb(wb, 0, [[64, 64], [b, nv], [1, b]])
            for half in range(2):
                dst = sb(wbpad, half * b, [[128, 64], [2 * b, nv], [1, b]])
                nc.scalar.copy(out=dst, in_=src)
            for u in range(nv):
                rin = sb(wbpad, u * b * 128, [[2 * b, nv], [1, b], [129, b]])
                rout = sb(d_raw, u * 64, [[b, nv], [1, b]])
                nc.vector.tensor_reduce(out=rout, in_=rin, axis=mybir.AxisListType.X,
                                        op=mybir.AluOpType.add)
            dsrc = sb(d_raw, 0, [[64, nv], [b, nv], [1, b]])
            for half in range(2):
                ddst = sb(dpad, half * b, [[128, nv], [2 * b, nv], [1, b]])
                nc.vector.tensor_scalar(out=ddst, in0=dsrc, scalar1=a_bc[:, idx:idx + 1],
                                        scalar2=1.0 / b, op0=mybir.AluOpType.mult,
                                        op1=mybir.AluOpType.mult)
            for u in range(nv):
                aout = sb(acc, u * b * 64, [[b, nv], [64, b], [1, b]])
                din = sb(dpad, u * 128 + b, [[2 * b, nv], [-1, b], [1, b]])
                nc.vector.tensor_tensor(out=aout, in0=aout, in1=din, op=mybir.AluOpType.add)

        # scatter ACC -> w_loc (bf16 natural rows, SWDGE cast), then AllGather
        wloc_4d = w_loc.ap().rearrange("(q r) (p s) -> q p r s", r=64, s=64)
        for q in range(NQ64):
            nc.gpsimd.dma_start(out=wloc_4d[q], in_=acc[q * 32:(q + 1) * 32, :])
        nc.gpsimd.collective_compute(
            "AllGather", mybir.AluOpType.bypass,
            replica_groups=[list(range(N_CORES))],
            ins=[w_loc.ap().opt()], outs=[w_full.ap().opt()],
        )

        # ---------- transposed weight reload: wT[ic] [128 i, 2048 o] ----------
        wT = []
        for ic in range(N_IC):
            t = nc.alloc_sbuf_tensor(f"wT{ic}", [128, OUT_F], bf16).ap()
            nc.sync.dma_start(out=t, in_=w_full.ap()[:, ic * 128:(ic + 1) * 128], transpose=True)
            wT.append(t)

        # ---------- x cast f32 -> bf16 (DRAM -> DRAM) ----------
        for g in range(4):
            nc.gpsimd.dma_start(out=xbf.ap()[g * 512:(g + 1) * 512, :],
                                in_=xs.ap()[g * 512:(g + 1) * 512, :])

        # ---------- main matmul ----------
        with (
            tc.tile_pool(name="xt", bufs=2) as xt_pool,
            tc.tile_pool(name="psum", bufs=2, space="PSUM") as psum_pool,
            tc.tile_pool(name="osb", bufs=4) as osb_pool,
        ):
            for tg in range(N_TG):
                xT = []
                for ic in range(N_IC):
                    t = xt_pool.tile([128, 512], bf16, name=f"xt{ic}")
                    nc.sync.dma_start(out=t[:],
                                      in_=xbf.ap()[tg * 512:(tg + 1) * 512, ic * 128:(ic + 1) * 128],
                                      transpose=True)
                    xT.append(t)
                for tsub in range(4):
                    psums = [psum_pool.tile([128, 512], f32, name=f"ps{o}", tag=f"ps{o}") for o in range(N_O)]
                    for ic in range(N_IC):
                        lhsT = xT[ic][:, tsub * 128:(tsub + 1) * 128]
                        for o in range(N_O):
                            nc.tensor.matmul(psums[o][:], lhsT, wT[ic][:, o * 512:(o + 1) * 512],
                                             start=(ic == 0), stop=(ic == N_IC - 1))
                    trow = tg * 512 + tsub * 128
                    for o in range(N_O):
                        ot = osb_pool.tile([128, 512], f32, name="ot")
                        nc.vector.tensor_tensor(out=ot[:], in0=psums[o][:],
                                                in1=bias_bc[:, o * 512:(o + 1) * 512],
                                                op=mybir.AluOpType.add)
                        nc.scalar.dma_start(out=out.ap()[trow:trow + 128, o * 512:(o + 1) * 512],
                                            in_=ot[:])

    nc.compile()
    return nc


def kernel(x, weight, bias, alphas, gumbels):
    if "nc" not in _CACHE:
        _CACHE["nc"] = _build_nc()
    nc = _CACHE["nc"]

    x2 = np.ascontiguousarray(np.asarray(x, np.float32).reshape(TOK_TOTAL, IN_F))
    weight = np.asarray(weight, np.float32)
    in_maps = []
    for c in range(N_CORES):
        in_maps.append({
            "xs": np.ascontiguousarray(x2[c * TOK:(c + 1) * TOK]),
            "ws": np.ascontiguousarray(weight[c * ROWS:(c + 1) * ROWS]),
            "bias": np.asarray(bias, np.float32).reshape(1, OUT_F),
            "alphas": np.asarray(alphas, np.float32).reshape(1, 7),
            "gumbels": np.asarray(gumbels, np.float32).reshape(1, 7),
        })
    res = run_bass_kernel_spmd(nc, in_maps, core_ids=list(range(N_CORES)))
    outs = [res.results[c]["out"] for c in range(N_CORES)]
    return np.concatenate(outs, axis=0).reshape(BATCH, TOKENS, OUT_F)
